# revision 26
# baseline (speedup 1.0000x reference)
"""CoAttenBlock Trainium2 kernel.

Full inputs in, full outputs out. Shards batch (B=8) across 8 NeuronCores,
one sample per core (pure data parallel, no collectives).

Per-core math (C=64, HW=2304, strips of 128 along the left position axis n):
  XL = WL @ [xlh;xll] + bL                      [64, 2304]
  XR = WR @ [xrh;xrl] + bR
  per strip s: aff_s = XL[:,s].T @ XR           [128, 2304]
               E_s   = exp(aff_s) -> bf16, rowsum via activation accum_out
               r2    = 1/rowsum  (folded into the strip's YRT weight columns)
               P12  += [YLT_s | YRT_s*r2].T @ E_s   (PSUM-resident [128, 2304])
  where YLT = (WLo_r @ XL).T strips, YRT = (WRo_r @ XR).T strips, so that
  P1 = WLo_r @ (XL @ E) and P2 = WRo_r @ (XR @ diag(r2) @ E).
  Gate pre-acts are recovered as vL.T @ P1 with vL = solve(WLo_r.T, gwL.T)
  (host-side 64x64 solve; inputs are deterministic, conditioning ~5e2).
  E/Wc/colsum run in bf16: bacc matmuls stay 1 cycle/row and the single
  colsum accumulator gets DVE's 2x two-byte mode; the ~0.5% bf16
  accumulation error only touches the sigmoid gates (budget 2e-2).
  All affs of a strip are emitted before the lagged bacc matmuls so a bacc
  stall (waiting on the r2 scale) can never block the aff->exp stream on
  the in-order PE queue.
  Tail gate math runs in a transposed [128,18] layout (column m = 128*mt+mp
  lives at partition mp, col mt) so ACT/DVE ops are 18 cols wide, then a
  bf16 PE transpose + SBUF->SBUF DMA rebuilds the row form for the
  ones-broadcast matmuls:
  s1 = sigmoid(g1pre*r1 + gb1)*r1, r1 = 1/colsum ; s2 = sigmoid(g2pre+gb2)
  out_L = WLo_l @ XLa + P1*s1[m] ; out_R = WRo_l @ XRa + P2*s2[m]
  (bLo/bRo folded in via an augmented ones row 64 on XLa/XRa).

float32r (single-pass PE mode) is used for the aff/conv/out matmuls;
producers of f32r-matmul inputs write with a f32r-typed output AP so the
engine rounds on write (BIR verifier requirement).
"""

import os
import sys

import numpy as np

if os.path.isdir("/opt/trn_rl_repo") and "/opt/trn_rl_repo" not in sys.path:
    sys.path.insert(0, "/opt/trn_rl_repo")

import concourse.bass as bass
import concourse.tile as tile
from concourse import bacc, mybir
from concourse.bass_utils import run_bass_kernel_spmd

B, C, H, W = 8, 64, 48, 48
HW = H * W            # 2304
C2 = 2 * C            # 128
NSTRIP = HW // 128    # 18
F32 = mybir.dt.float32
F32R = mybir.dt.float32r
BF16 = mybir.dt.bfloat16
AF = mybir.ActivationFunctionType


def chunks(total, step=512):
    out = []
    c0 = 0
    while c0 < total:
        out.append((c0, min(step, total - c0)))
        c0 += step
    return out


CH_2304 = chunks(2304)            # 4x512 + 256

# f32 weight-pack column layout
PK_WLT = slice(0, 64)
PK_WRT = slice(64, 128)
PK_WLORT = slice(128, 192)
PK_WRORT = slice(192, 256)
PK_WLOLTA = slice(256, 320)
PK_WROLTA = slice(320, 384)
PK_VLR2 = slice(384, 386)
PK_GBL = slice(386, 387)
PK_GBR = slice(387, 388)
PK_BLROW = slice(388, 452)
PK_BRCOL = slice(452, 453)
PKF_COLS = 453
# bf16 pack: id128 [0:128], ones64 rows0/64 [128:192], ones col [192], id64 [196:260]
PKB_COLS = 260


def r(ap):
    return ap.bitcast(F32R)


def build_nc():
    nc = bacc.Bacc("TRN2", target_bir_lowering=False, debug=False)

    x2l_d = nc.dram_tensor("x2l", [C2, HW], F32, kind="ExternalInput").ap()
    x2r_d = nc.dram_tensor("x2r", [C2, HW], F32, kind="ExternalInput").ap()
    pkf_d = nc.dram_tensor("pkf", [C2, PKF_COLS], F32, kind="ExternalInput").ap()
    pkb_d = nc.dram_tensor("pkb", [C2, PKB_COLS], BF16, kind="ExternalInput").ap()
    ones_np = np.ones((1, HW), dtype=np.float32)
    ones_d = nc.inline_tensor(ones_np, "onesrow").ap()

    out_l_d = nc.dram_tensor("out_l", [C, HW], F32, kind="ExternalOutput").ap()
    out_r_d = nc.dram_tensor("out_r", [C, HW], F32, kind="ExternalOutput").ap()

    with tile.TileContext(nc) as tc:
        import contextlib

        with contextlib.ExitStack() as outer:
            consts = outer.enter_context(tc.tile_pool(name="consts", bufs=1))
            big = outer.enter_context(tc.tile_pool(name="big", bufs=1))
            epool = outer.enter_context(tc.tile_pool(name="epool", bufs=6))
            smalls = outer.enter_context(tc.tile_pool(name="smalls", bufs=3))
            tailsb = outer.enter_context(tc.tile_pool(name="tailsb", bufs=1))

            pk = consts.tile([C2, PKF_COLS], F32)
            pkb = consts.tile([C2, PKB_COLS], BF16)

            # ---- big SBUF tensors ----
            x2l = big.tile([C2, HW], F32)
            x2r = big.tile([C2, HW], F32)
            XLa = big.tile([C + 1, HW], F32)   # row 64 = ones (bias fold)
            XRa = big.tile([C + 1, HW], F32)
            Wc = big.tile([C2, HW], BF16)      # 18 strips of [YLT | YRT]
            csum = big.tile([C2, HW], BF16)    # colsum accumulator (DVE 2x)
            P12sb = big.tile([C2, HW], F32)    # drained P1 (0:64) / P2 (64:128)
            outLR = big.tile([C2, HW], F32)

            # Input DMA order tuned for the phase-1 pipeline: weights, then
            # chunks in first-use order. ones rows + bf16 pack go through the
            # ACT-issued queue in parallel with the SP queue.
            LCH = [(0, 768), (768, 768), (1536, 768)]
            RCH = [(0, 768), (768, 768), (1536, 768)]
            nc.sync.dma_start(out=r(pk), in_=r(pkf_d))
            nc.gpsimd.dma_start(out=r(XLa[C:C + 1, :]), in_=r(ones_d))
            nc.gpsimd.dma_start(out=r(XRa[C:C + 1, :]), in_=r(ones_d))
            nc.gpsimd.dma_start(out=pkb, in_=pkb_d)

            def dma_in(dst, src, c0, cn):
                nc.sync.dma_start(out=r(dst[:, c0:c0 + cn]),
                                  in_=r(src[:, c0:c0 + cn]))

            dma_in(x2l, x2l_d, *LCH[0])
            dma_in(x2r, x2r_d, *RCH[0])
            dma_in(x2r, x2r_d, *RCH[1])
            dma_in(x2l, x2l_d, *LCH[1])
            dma_in(x2r, x2r_d, *RCH[2])
            dma_in(x2l, x2l_d, *LCH[2])

            wlT = r(pk[:, PK_WLT])
            wrT = r(pk[:, PK_WRT])
            wloRT = r(pk[0:C, PK_WLORT])
            wroRT = r(pk[0:C, PK_WRORT])
            wloLTa = r(pk[0:C + 1, PK_WLOLTA])
            wroLTa = r(pk[0:C + 1, PK_WROLTA])
            vlr2f = pk[:, PK_VLR2]
            gbL = pk[:, PK_GBL]
            gbR = pk[:, PK_GBR]
            bLrow = r(pk[C:C + 1, PK_BLROW])
            bRcol = pk[0:C, PK_BRCOL]
            id128b = pkb[:, 0:128]
            ones64b = pkb[:, 128:192]
            onecolb = pkb[:, 192:193]
            id64b = pkb[0:C, 196:260]

            with contextlib.ExitStack() as ph2_psum:
                p12p = ph2_psum.enter_context(
                    tc.tile_pool(name="p12p", bufs=1, space="PSUM"))
                affp = ph2_psum.enter_context(
                    tc.tile_pool(name="affp", bufs=1, space="PSUM"))
                P12 = p12p.tile([C2, HW], F32)  # lives all of phase 1+2
                ring = affp.tile([C2, 1536], F32, tag="ring", name="aff_ring")

                # ---- phase 1 pieces: conv scratch lives inside P12
                # (rows 0:64 for L, 64:128 for R; bacc starts 2 strips in) ----
                def conv_L(j):
                    c0, cn = CH_2304[j]
                    sl = slice(c0, c0 + cn)
                    nc.tensor.matmul(P12[0:C, sl], wlT, r(x2l[:, sl]),
                                     start=True, stop=False)
                    nc.tensor.matmul(P12[0:C, sl], bLrow, r(XLa[C:C + 1, sl]),
                                     start=False, stop=True)
                    nc.vector.tensor_copy(r(XLa[0:C, sl]), P12[0:C, sl])

                def conv_R(j):
                    c0, cn = CH_2304[j]
                    sl = slice(c0, c0 + cn)
                    rsl = (j % 3) * 512
                    nc.tensor.matmul(ring[0:C, rsl:rsl + cn], wrT, r(x2r[:, sl]),
                                     start=True, stop=True)
                    nc.scalar.activation(r(XRa[0:C, sl]), ring[0:C, rsl:rsl + cn],
                                         AF.Identity, bias=bRcol, scale=1.0)

                def emit_y(t, drain_eng):
                    # Wc strip t = [YLT_t | YRT_t]; PSUM scratch inside P12
                    # (pre-bacc window only: strips 0-1 of the loop).
                    nc.tensor.matmul(P12[:, 128 * t:128 * t + 64],
                                     r(XLa[0:C, 128 * t:128 * t + 128]),
                                     wloRT, start=True, stop=True)
                    nc.tensor.matmul(P12[:, 128 * t + 64:128 * t + 128],
                                     r(XRa[0:C, 128 * t:128 * t + 128]),
                                     wroRT, start=True, stop=True)
                    dst = Wc[:, 128 * t:128 * t + 128]
                    srcp = P12[:, 128 * t:128 * t + 128]
                    if drain_eng == "dve":
                        nc.vector.tensor_copy(dst, srcp)
                    else:
                        nc.scalar.copy(dst, srcp)

                phase = 0
                r2s = {}

                def emit_bacc(sb, c0, cn):
                    nc.tensor.matmul(P12[:, c0:c0 + cn],
                                     Wc[:, 128 * sb:128 * sb + 128],
                                     Es[sb][:, c0:c0 + cn],
                                     start=(sb == 0), stop=(sb == NSTRIP - 1))

                def emit_csum(sb):
                    if sb == 0:
                        nc.vector.tensor_copy(csum, Es[sb])
                    else:
                        nc.vector.tensor_add(csum, csum, Es[sb])

                # schedules for interleaving phase-1 work into strips 0/1:
                # section i of strip 0/1 emits these conv/emit_y pieces.
                L_SCHED = {0: {0: [0], 5: [1], 8: [2]}, 1: {0: [3], 2: [4]}}
                R_SCHED = {0: {0: [0], 2: [1], 4: [2], 6: [3], 8: [4]}, 1: {}}

                def emit_A(s, phase, sb):
                    # scale strip sb's YRT by r2, affs+exps for strip s, then
                    # the lagged bacc matmuls for sb (after ALL affs so a bacc
                    # stall can't block the exp stream on in-order PE).
                    if sb >= 0:
                        wright = Wc[:, 128 * sb + 64:128 * sb + 128]
                        nc.vector.tensor_scalar_mul(wright, wright, r2s[sb])
                        emit_csum(sb)
                    E = epool.tile([C2, HW], BF16, tag="e", name=f"E_{s}")
                    rs = smalls.tile([C2, 4], F32, tag="rs", name=f"rs_{s}")
                    lhs_aff = r(XLa[0:C, 128 * s:128 * s + 128])
                    pieces = [(256 * i, 256, (phase + i) % 6 * 256)
                              for i in range(9)]
                    groups = []
                    for p0, pn, ra in pieces:
                        if groups and groups[-1][2] + groups[-1][1] == ra \
                                and groups[-1][1] + pn <= 1024:
                            groups[-1][1] += pn
                        else:
                            groups.append([p0, pn, ra])
                    gidx = 0
                    done = 0
                    for i, (p0, pn, ra) in enumerate(pieces):
                        for j in L_SCHED.get(s, {}).get(i, []):
                            conv_L(j)
                        for j in R_SCHED.get(s, {}).get(i, []):
                            conv_R(j)
                        affdst = ring[:, ra:ra + pn]
                        nc.tensor.matmul(affdst,
                                         lhs_aff, r(XRa[0:C, p0:p0 + pn]),
                                         start=True, stop=True)
                        done += pn
                        while gidx < len(groups) and \
                                groups[gidx][0] + groups[gidx][1] <= done:
                            m0, mn, r0 = groups[gidx]
                            expsrc = ring[:, r0:r0 + mn]
                            if gidx == len(groups) - 1:
                                # mini-slot group: plain exp, slot frees at
                                # exp end; its rowsum share comes from a DVE
                                # reduce over the bf16 E instead.
                                nc.scalar.activation(E[:, m0:m0 + mn],
                                                     expsrc, AF.Exp)
                            else:
                                nc.scalar.activation(E[:, m0:m0 + mn],
                                                     expsrc, AF.Exp,
                                                     accum_out=rs[:, gidx:gidx + 1])
                            gidx += 1
                        if s == 0 and 1 <= i <= 8:
                            emit_y(i - 1, "dve")
                        if s == 1 and i <= 8:
                            for t in (8 + i,) if i < 8 else (16, 17):
                                emit_y(t, "dve")
                    ng = len(groups)
                    m0l, mnl, _ = groups[-1]
                    nc.vector.tensor_reduce(rs[:, ng - 1:ng],
                                            E[:, m0l:m0l + mnl],
                                            axis=mybir.AxisListType.X,
                                            op=mybir.AluOpType.add)
                    if sb >= 0:
                        for bc0, bcn in CH_2304:
                            emit_bacc(sb, bc0, bcn)
                    rowsum = smalls.tile([C2, 1], F32, tag="rowsum",
                                         name=f"rowsum_{s}")
                    nc.vector.tensor_reduce(rowsum, rs[:, 0:ng],
                                            axis=mybir.AxisListType.X,
                                            op=mybir.AluOpType.add)
                    r2 = smalls.tile([C2, 1], F32, tag="r2", name=f"r2_{s}",
                                     bufs=4)
                    nc.vector.reciprocal(r2, rowsum)
                    r2s[s] = r2
                    return E

                def emit_B_tail(sb):
                    wright = Wc[:, 128 * sb + 64:128 * sb + 128]
                    nc.vector.tensor_scalar_mul(wright, wright, r2s[sb])
                    # csum first: the tail gate chain waits on the full colsum,
                    # so get it onto DVE before the P12 drains queue up.
                    emit_csum(sb)
                    drain_eng = ["dve", "act", "dve", "act", "dve"]
                    for j, (c0, cn) in enumerate(CH_2304):
                        emit_bacc(sb, c0, cn)
                        if sb == NSTRIP - 1:
                            # P12 piece is final once the last strip's bacc
                            # for it retires: drain immediately.
                            sl = slice(c0, c0 + cn)
                            if drain_eng[j] == "act":
                                nc.scalar.copy(r(P12sb[:, sl]), P12[:, sl])
                            else:
                                nc.vector.tensor_copy(r(P12sb[:, sl]), P12[:, sl])

                Es = {}
                for s in range(NSTRIP):
                    Es[s] = emit_A(s, phase, s - 2)
                    phase = (phase + 9) % 6
                for s in (NSTRIP - 2, NSTRIP - 1):
                    emit_B_tail(s)

            # ---- phase 3: transposed gate chain + broadcast/gate/out ----
            with tc.tile_pool(name="ph3p", bufs=1, space="PSUM") as ph3:
                # one shared bank for all the small tail tiles
                tailps = ph3.tile([C2, 512], F32)
                cst = tailps[:, 0:NSTRIP]
                gpt = tailps[:, 32:32 + 2 * NSTRIP]
                s12T = tailps[0:2 * NSTRIP, 128:192].bitcast(BF16)  # [36,128]
                # colsum transposed: cst[mp, mt] = sum_p csum[p, 128*mt+mp]
                for mt in range(NSTRIP):
                    m0 = 128 * mt
                    nc.tensor.matmul(cst[:, mt:mt + 1],
                                     csum[:, m0:m0 + 128], onecolb,
                                     start=True, stop=True)
                    # gate pre-acts transposed, both sides at once
                    nc.tensor.matmul(gpt[:, 2 * mt:2 * mt + 2],
                                     P12sb[:, m0:m0 + 128], vlr2f,
                                     start=True, stop=True)
                r1t = tailsb.tile([C2, NSTRIP], F32)
                nc.vector.reciprocal(r1t, cst)
                g1pre = tailsb.tile([C2, NSTRIP], F32)
                nc.vector.tensor_mul(g1pre, gpt[:, 0::2], r1t)
                g1t = tailsb.tile([C2, NSTRIP], F32)
                s12j = tailsb.tile([C2, 2 * NSTRIP], BF16)
                nc.scalar.activation(g1t, g1pre, AF.Sigmoid, bias=gbL, scale=1.0)
                nc.scalar.activation(s12j[:, NSTRIP:], gpt[:, 1::2], AF.Sigmoid,
                                     bias=gbR, scale=1.0)
                nc.vector.tensor_mul(s12j[:, 0:NSTRIP], g1t, r1t)
                # transpose to rows, rebuild [rows 0/64, 2304] layout via two
                # SBUF->SBUF DMAs on separate issue queues
                nc.tensor.transpose(s12T, s12j, id128b)
                s12Ts = tailsb.tile([2 * NSTRIP, 128], BF16)
                nc.vector.tensor_copy(s12Ts, s12T)
                srow = tailsb.tile([C + 1, HW], BF16)  # s1 row 0, s2 row 64
                nc.sync.dma_start(out=srow[0:1, :], in_=s12Ts[0:NSTRIP, :])
                nc.scalar.dma_start(out=srow[C:C + 1, :], in_=s12Ts[NSTRIP:, :])

                # per-chunk: S broadcast, gate-mul, out conv + id-add, drain
                for q, (p0, pn) in enumerate(CH_2304):
                    sl = slice(p0, p0 + pn)
                    for side in (0, 1):
                        rows = slice(0, C) if side == 0 else slice(C, C2)
                        S = ph3.tile([C, pn], F32, tag=f"S{side}",
                                     name=f"S{side}_{q}", padded_shape=[C, 512],
                                     bufs=2)
                        srow_r = srow[0:1, sl] if side == 0 else srow[C:C + 1, sl]
                        ones_r = ones64b[0:1, :] if side == 0 else ones64b[C:C + 1, :]
                        nc.tensor.matmul(S, ones_r, srow_r,
                                         start=True, stop=True)
                        t_ = tailsb.tile([C, pn], BF16, tag=f"t{side}",
                                         name=f"t{side}_{q}",
                                         padded_shape=[C, 512], bufs=2)
                        nc.vector.tensor_mul(t_, P12sb[rows, sl], S)
                        O = ph3.tile([C, pn], F32, tag=f"O{side}",
                                     name=f"O{side}_{q}", padded_shape=[C, 512])
                        Xa = XLa if side == 0 else XRa
                        wA = wloLTa if side == 0 else wroLTa
                        nc.tensor.matmul(O, wA, r(Xa[:, sl]),
                                         start=True, stop=False)
                        nc.tensor.matmul(O, id64b, t_,
                                         start=False, stop=True)
                        nc.scalar.copy(r(outLR[rows, sl]), O)
                    if p0 + pn in (1024, 2048):
                        d0 = p0 + pn - 1024
                        nc.sync.dma_start(out=out_l_d[:, d0:p0 + pn],
                                          in_=outLR[0:C, d0:p0 + pn])
                        nc.sync.dma_start(out=out_r_d[:, d0:p0 + pn],
                                          in_=outLR[C:C2, d0:p0 + pn])
                nc.sync.dma_start(out=out_l_d[:, 2048:], in_=outLR[0:C, 2048:])
                nc.sync.dma_start(out=out_r_d[:, 2048:], in_=outLR[C:C2, 2048:])

    nc.compile()
    return nc


_NC_CACHE = {}


def _get_nc():
    if "nc" not in _NC_CACHE:
        _NC_CACHE["nc"] = build_nc()
    return _NC_CACHE["nc"]


def _prep_shared(concaL_w, concaL_b, concaR_w, concaR_b,
                 gateL_w, gateL_b, gateR_w, gateR_b,
                 concaLo_w, concaLo_b, concaRo_w, concaRo_b):
    f = np.float32
    wloR = np.asarray(concaLo_w)[:, C:].astype(np.float64)
    wroR = np.asarray(concaRo_w)[:, C:].astype(np.float64)
    vL = np.linalg.solve(wloR.T, np.asarray(gateL_w).astype(np.float64).reshape(C))
    vR = np.linalg.solve(wroR.T, np.asarray(gateR_w).astype(np.float64).reshape(C))

    pkf = np.zeros((C2, PKF_COLS), dtype=f)
    pkf[:, PK_WLT] = np.asarray(concaL_w).T
    pkf[:, PK_WRT] = np.asarray(concaR_w).T
    pkf[0:C, PK_WLORT] = wloR.T
    pkf[0:C, PK_WRORT] = wroR.T
    pkf[0:C, PK_WLOLTA] = np.asarray(concaLo_w)[:, :C].T
    pkf[C, PK_WLOLTA] = np.asarray(concaLo_b).reshape(C)
    pkf[0:C, PK_WROLTA] = np.asarray(concaRo_w)[:, :C].T
    pkf[C, PK_WROLTA] = np.asarray(concaRo_b).reshape(C)
    pkf[0:C, 384] = vL
    pkf[C:C2, 385] = vR
    pkf[:, 386] = np.asarray(gateL_b).reshape(())
    pkf[:, 387] = np.asarray(gateR_b).reshape(())
    pkf[C, PK_BLROW] = np.asarray(concaL_b).reshape(C)
    pkf[0:C, PK_BRCOL] = np.asarray(concaR_b).reshape(C, 1)

    pkb = np.zeros((C2, PKB_COLS), dtype=np.float32)
    pkb[:, 0:128] = np.eye(C2, dtype=f)
    pkb[0, 128:192] = 1.0
    pkb[C, 128:192] = 1.0
    pkb[:, 192] = 1.0
    pkb[0:C, 196:260] = np.eye(C, dtype=f)
    import jax.numpy as jnp
    pkb16 = np.asarray(jnp.asarray(pkb, dtype=jnp.bfloat16))

    return {"pkf": np.ascontiguousarray(pkf),
            "pkb": np.ascontiguousarray(pkb16)}


def kernel(xlh, xll, xrh, xrl,
           concaL_w, concaL_b, concaR_w, concaR_b,
           gateL_w, gateL_b, gateR_w, gateR_b,
           concaLo_w, concaLo_b, concaRo_w, concaRo_b,
           _return_results=False):
    nc = _get_nc()
    shared = _prep_shared(concaL_w, concaL_b, concaR_w, concaR_b,
                          gateL_w, gateL_b, gateR_w, gateR_b,
                          concaLo_w, concaLo_b, concaRo_w, concaRo_b)
    xlh = np.asarray(xlh, dtype=np.float32)
    xll = np.asarray(xll, dtype=np.float32)
    xrh = np.asarray(xrh, dtype=np.float32)
    xrl = np.asarray(xrl, dtype=np.float32)

    in_maps = []
    for c in range(B):
        x2l = np.concatenate([xlh[c].reshape(C, HW), xll[c].reshape(C, HW)], axis=0)
        x2r = np.concatenate([xrh[c].reshape(C, HW), xrl[c].reshape(C, HW)], axis=0)
        m = dict(shared)
        m["x2l"] = np.ascontiguousarray(x2l)
        m["x2r"] = np.ascontiguousarray(x2r)
        in_maps.append(m)

    # The first execution of a freshly compiled NEFF occasionally hits a
    # transient NRT_EXEC_UNIT_UNRECOVERABLE on this axon setup; an immediate
    # re-dispatch of the same executable has always succeeded, so retry.
    res = None
    for attempt in range(3):
        try:
            res = run_bass_kernel_spmd(nc, in_maps, list(range(B)))
            break
        except Exception:
            if attempt == 2:
                raise
            import time as _time
            _time.sleep(2.0)
    out_L = np.stack([res.results[c]["out_l"].reshape(C, H, W) for c in range(B)])
    out_R = np.stack([res.results[c]["out_r"].reshape(C, H, W) for c in range(B)])
    if _return_results:
        return (out_L, out_R), res
    return (out_L, out_R)
